# revision 25
# baseline (speedup 1.0000x reference)
"""Causal multi-head self-attention on 8 Trainium2 NeuronCores.

Sharding: core c = (b, g) with b = c // 4 (batch), g = c % 4 (head group).
Each core computes 4 of the 16 heads for one batch element:
  Q/K/V projections for feature rows 256g:256g+256 (Megatron column split),
  causal attention for those heads, and a partial output projection
  against Wo[:, 256g:256g+256] (row split). Host sums the 4 partials per batch.

All operands are pre-transposed on the host so the kernel never transposes:
  xt  = X[b].T          [D, S]   (d on partitions -> matmul contraction dim)
  wqt = Wq[rows].T      [D, 256]
  wkt = Wk[rows].T      [D, 256]
  wvt = Wv[rows].T      [D, 256]
  wot = Wo[:, cols].T   [256, D]

Attention is computed with scores transposed (S^T = K Q^T, kv on partitions)
so the PV matmul needs no transpose, and a ones-row appended to V yields the
softmax denominator inside the same accumulation.

v4 structure (changes from v3):
  - causal diagonal trimming: for the diagonal kv chunk at block-relative
    position j, the scores matmuls, exp, and PV matmuls only cover
    q >= 128j (the fully-masked prefix is never computed or read).
  - the triangle mask is applied as a DVE multiply against a constant
    [128,2,128] 0/1 tile instead of a gpsimd affine_select: gpsimd now only
    runs the per-pair reciprocal broadcast, so its strict FIFO can no longer
    convoy the PE behind norm chains (v3 lost ~12us + two HAM re-throttles
    to this).
  - softmax normalization is batched per head-pair: PV accumulates both
    heads into one [65,2,512] psum (2 banks), a single evac / denominator
    copy / reciprocal / partition_broadcast covers both heads, then two
    muls write ot_t. Denominator staged through partition 0 (custom DVE
    ops break on nonzero base partitions on HW).
  - psum budget: scores 2x2 banks double-buffered, po 2 banks single, proj
    [128,512] double-buffered (projection/output chains no longer stall on
    a single bank between chain and evac).
  - dummy warm matmuls cover the final norm chain so the tail output
    projection does not run at the HAM-throttled 1.2 GHz clock.
  - partial outputs are stored and DMA'd as fp16 (host sums in fp32),
    halving output traffic and the post-compute DMA tail.
"""

import sys

sys.path.insert(0, "/opt/trn_rl_repo")

import numpy as np

B = 2
S = 2048
D = 1024
H = 16
DH = 64

NCORES = 8
GROUPS = 4            # head groups (cores per batch element)
HPC = H // GROUPS     # heads per core = 4
F = HPC * DH          # feature slice per core = 256

_nc_cache = {}


def _build(s=S):
    import concourse.bass as bass  # noqa: F401
    import concourse.mybir as mybir
    import concourse.tile as tile
    from concourse import bacc

    f32 = mybir.dt.float32
    f16 = mybir.dt.float16
    bf16 = mybir.dt.bfloat16
    dmm = bf16  # matmul operand dtype

    P = 128
    SB = 512               # q-block / free-dim block
    NSB = s // SB          # q blocks
    KC = D // P            # 8 contraction chunks over D
    KH = KC // 2           # half-chain k chunks
    MC = F // P            # 2 feature chunks per core
    NSC = s // P           # s chunks of 128
    ND = D // SB           # 2 output column blocks
    NWARM = 20             # PE warmup matmuls during initial DMA wait
    NDUM = 18              # PE keep-warm matmuls over the final norm chain
    LAG = 2                # kv chunks the PV matmul trails the exp by

    nc = bacc.Bacc("TRN2", debug=False, num_devices=NCORES)
    # all inputs are pre-tiled on the host into [partition, ...] layouts so
    # every DMA reads contiguous 2-8KB per partition (the natural
    # rearrange patterns gave 0.5-1KB scattered segments and only
    # ~120-200 GB/s, starving the projection phase)
    xt = nc.dram_tensor("xt", [P, NSB, KC, SB], dmm, kind="ExternalInput").ap()
    wqt = nc.dram_tensor("wqt", [P, KC, F], dmm, kind="ExternalInput").ap()
    wkt = nc.dram_tensor("wkt", [P, KC, F], dmm, kind="ExternalInput").ap()
    wvt = nc.dram_tensor("wvt", [P, KC, F], dmm, kind="ExternalInput").ap()
    wot = nc.dram_tensor("wot", [P, MC, D], dmm, kind="ExternalInput").ap()
    y = nc.dram_tensor("y", [s, D], f16, kind="ExternalOutput").ap()

    with tile.TileContext(nc) as tc:
        with (
            tc.tile_pool(name="w", bufs=1) as wpool,
            tc.tile_pool(name="const", bufs=1) as cpool,
            tc.tile_pool(name="xt", bufs=2) as xpool,
            tc.tile_pool(name="qkv", bufs=1) as qkvpool,
            tc.tile_pool(name="pt", bufs=4) as ptpool,
            tc.tile_pool(name="small", bufs=4) as spool,
            tc.tile_pool(name="yst", bufs=3) as ypool,
            tc.tile_pool(name="ps", bufs=1, space="PSUM") as pspool,
        ):
            wq_s = wpool.tile([P, KC, F], dmm, name="wq_s")
            wk_s = wpool.tile([P, KC, F], dmm, name="wk_s")
            wv_s = wpool.tile([P, KC, F], dmm, name="wv_s")
            wo_s = wpool.tile([P, MC, D], dmm, name="wo_s")
            wqt_r = wqt
            wkt_r = wkt
            # all input DMAs posted upfront, in first-needed order, so a
            # late-posted transfer can never stall the in-order PE queue
            xt_tiles = [
                xpool.tile([P, KC, SB], dmm, name="xt_tile", bufs=4)
                for _ in range(NSB)
            ]
            # warm_w memset first on gpsimd so PE warmup starts the moment
            # the prologue ends (nothing queued ahead of it)
            warm_w = cpool.tile([P, SB], dmm, name="warm_w")
            nc.gpsimd.memset(warm_w[:], 0.0)

            # input DMAs on the sync queue in first-needed order; the two
            # late xt tiles ride the scalar queue so they never delay wo
            nc.sync.dma_start(wq_s[:, 0:KH, :], wqt_r[:, 0:KH, :])
            nc.sync.dma_start(wk_s[:, 0:KH, :], wkt_r[:, 0:KH, :])
            nc.sync.dma_start(xt_tiles[0][:, 0:KH, :], xt[:, 0, 0:KH, :])
            nc.sync.dma_start(wq_s[:, KH:KC, :], wqt_r[:, KH:KC, :])
            nc.sync.dma_start(wk_s[:, KH:KC, :], wkt_r[:, KH:KC, :])
            nc.sync.dma_start(xt_tiles[0][:, KH:KC, :], xt[:, 0, KH:KC, :])
            nc.sync.dma_start(wv_s[:], wvt[:])
            nc.sync.dma_start(xt_tiles[1][:], xt[:, 1, :, :])
            # wo is first needed by ph3(0), woven into attn(1) - well after
            # xt1 (which gates proj(1) right behind proj(0))
            nc.sync.dma_start(wo_s[:], wot[:])
            for sb in range(2, NSB):
                nc.scalar.dma_start(
                    xt_tiles[sb][:], xt[:, sb, :, :]
                )

            # --- PE warmup: dummy matmuls on memset data so the HAM clock
            # gate reaches 8/8 before the first real (DMA-gated) matmul ---
            for i in range(NWARM):
                wps = pspool.tile([P, SB], f32, name="wps", tag="proj", bufs=2)
                nc.tensor.matmul(
                    wps[:], warm_w[:, 0:P], warm_w[:], start=True, stop=True
                )

            # --- constant triangle mask: mask3[r, slot, c] = 1 iff c >= r.
            # Applied to the diagonal 128-col window of pt by a DVE multiply
            # (stale columns left of the window are never read). ---
            mask3 = cpool.tile([P, 2, P], dmm, name="mask3")
            nc.gpsimd.memset(mask3[:], 1.0)
            nc.gpsimd.affine_select(
                out=mask3[:],
                in_=mask3[:],
                compare_op=mybir.AluOpType.is_ge,
                fill=0.0,
                base=0,
                pattern=[[0, 2], [1, P]],
                channel_multiplier=-1,
            )

            # --- persistent activations ---
            qt_t = qkvpool.tile([P, MC, s], dmm, name="qt_t")   # Q^T
            kt_t = qkvpool.tile([P, MC, s], dmm, name="kt_t")   # K^T
            v_t = qkvpool.tile([P, NSC, HPC, DH + 1], dmm, name="v_t")  # V | 1
            ot_t = qkvpool.tile([P, MC, s], dmm, name="ot_t")   # attn out ^T
            ones_sb = cpool.tile([P, NSC * HPC], f32, name="ones_sb")
            nc.gpsimd.memset(ones_sb[:], 1.0)
            nc.vector.tensor_copy(
                out=v_t[:, :, :, DH:DH + 1],
                in_=ones_sb.rearrange("p (a b) -> p a b", b=HPC)[:, :, :, None],
            )

            copy_alt = [0]

            def evac(dst, src, alternate):
                """psum -> sbuf evacuation, optionally alternating engines."""
                if alternate:
                    copy_alt[0] ^= 1
                    if copy_alt[0]:
                        nc.scalar.copy(dst, src)
                        return
                nc.vector.tensor_copy(out=dst, in_=src)

            def proj_chains(sb, alternate=False):
                """Projection block sb as a list of half-chain closures."""
                xt_tile = xt_tiles[sb]

                def qk_half(w_s, dst, m, kh, box):
                    def emit():
                        if kh == 0:
                            box["pp"] = pspool.tile(
                                [P, SB], f32, name="pp", tag="proj", bufs=2
                            )
                        pp = box["pp"]
                        for k in range(kh * KH, (kh + 1) * KH):
                            nc.tensor.matmul(
                                pp[:],
                                (w_s[:, k, m * P:(m + 1) * P]),
                                (xt_tile[:, k, :]),
                                start=(k == 0),
                                stop=(k == KC - 1),
                            )
                        if kh == 1:
                            evac(
                                dst[:, m, sb * SB:(sb + 1) * SB], pp[:],
                                alternate,
                            )
                    return emit

                def v_half(sc, kh, box):
                    def emit():
                        if kh == 0:
                            box["pv"] = pspool.tile(
                                [P, SB], f32, name="pv", tag="proj", bufs=2
                            )
                        pv = box["pv"]
                        for k in range(kh * KH, (kh + 1) * KH):
                            nc.tensor.matmul(
                                pv[:, :F],
                                (xt_tile[:, k, sc * P:(sc + 1) * P]),
                                (wv_s[:, k, :]),
                                start=(k == 0),
                                stop=(k == KC - 1),
                            )
                        if kh == 1:
                            nc.vector.tensor_copy(
                                out=v_t[:, sb * 4 + sc, :, 0:DH],
                                in_=pv[:, :F].rearrange(
                                    "p (h d) -> p h d", d=DH
                                ),
                            )
                    return emit

                qk = []
                for m in range(MC):
                    for w_s, dst in ((wq_s, qt_t), (wk_s, kt_t)):
                        box = {}
                        qk.append(qk_half(w_s, dst, m, 0, box))
                        qk.append(qk_half(w_s, dst, m, 1, box))
                vv = []
                for sc in range(SB // P):
                    box = {}
                    vv.append(v_half(sc, 0, box))
                    vv.append(v_half(sc, 1, box))
                return qk, vv

            def ph3_chains(qb, alternate, use_o_tag=False):
                """Output projection for q-block qb as chain closures. Both
                column blocks of a q-chunk evacuate into one [128, 1024] ys
                tile and ship as a single DMA (the ~600ns/instruction DMA
                issue rate on the sync queue otherwise throttles the tail).
                In the final block py rotates across the freed proj+s psum
                tags (4 banks) so the chains never wait on an evac."""
                def chain(sc, nb, box):
                    tag, bufs = ("s", 2) if use_o_tag and (sc * ND + nb) % 2 \
                        else ("proj", 2)

                    def emit():
                        py = pspool.tile(
                            [P, SB], f32, name="py", tag=tag, bufs=bufs
                        )
                        for o in range(MC):
                            nc.tensor.matmul(
                                py[:],
                                (ot_t[:, o, sc * P:(sc + 1) * P]),
                                (wo_s[:, o, nb * SB:(nb + 1) * SB]),
                                start=(o == 0),
                                stop=(o == MC - 1),
                            )
                        if nb == 0:
                            box["ys"] = ypool.tile(
                                [P, ND, SB], f16, name="ys", bufs=4
                            )
                        ys = box["ys"]
                        evac(ys[:, nb, :], py[:], alternate)
                        if nb == ND - 1:
                            nc.sync.dma_start(
                                y[sc * P:(sc + 1) * P, :],
                                ys[:],
                            )
                    return emit

                out = []
                for sc in range(4 * qb, 4 * qb + 4):
                    box = {}
                    for nb in range(ND):
                        out.append(chain(sc, nb, box))
                return out

            def attn(qb, fill, pre_norms, fill_front=()):
                """Attention for q-block qb; fill chains are woven into the
                exp-paced kv loop to keep the in-order PE queue busy.
                pre_norms are the deferred normalize stages of the previous
                head-pair/q-block: stage 1 (denominator copy + reciprocal +
                broadcast) fires at kv==1, stage 2 (the muls, which wait on
                the gpsimd broadcast) at kv==3 - so this block's mask
                multiplies are never queued in the DVE FIFO behind a mul
                that is itself blocked on gpsimd."""
                nkv = 4 * (qb + 1)
                final = qb == NSB - 1
                qsl = slice(qb * SB, (qb + 1) * SB)
                ncycles = (HPC // 2) * nkv
                cyc = 0
                nfill = 0
                nfront = 0
                pending = list(pre_norms)
                for hp in range(HPC // 2):
                    mo = hp
                    # both heads of the pair accumulate into one psum tile
                    # (2 banks); a single evac/recip/broadcast then serves
                    # both heads' normalization
                    po = pspool.tile(
                        [DH + 1, 2, SB], f32, name="po", tag="o", bufs=1
                    )
                    pts = []   # (pt tile, c0) per kv chunk
                    c0s = []

                    def pv_step(kv, stop):
                        pt, c0 = pts[kv]
                        for slot in (0, 1):
                            h = 2 * hp + slot
                            nc.tensor.matmul(
                                po[:, slot, c0:],
                                (v_t[:, kv, h, :]),
                                (pt[:, slot, c0:]),
                                start=(kv == 0),
                                stop=stop,
                            )

                    for kv in range(nkv):
                        j = kv - 4 * qb
                        c0 = P * j if j > 0 else 0
                        qs0 = qb * SB + c0
                        psS = pspool.tile(
                            [P, 2, SB], f32, name="psS", tag="s", bufs=2
                        )
                        # the two heads of the pair sit on different PE row
                        # groups (rows 0-63 / 64-127) -> concurrent matmuls
                        nc.tensor.matmul(
                            psS[:, 0, c0:],
                            (kt_t[0:DH, mo, kv * P:(kv + 1) * P]),
                            (qt_t[0:DH, mo, qs0:(qb + 1) * SB]),
                            start=True,
                            stop=True,
                        )
                        nc.tensor.matmul(
                            psS[:, 1, c0:],
                            (kt_t[DH:P, mo, kv * P:(kv + 1) * P]),
                            (qt_t[DH:P, mo, qs0:(qb + 1) * SB]),
                            start=True,
                            stop=True,
                        )
                        pt = ptpool.tile([P, 2, SB], dmm, name="pt", bufs=4)
                        nc.scalar.activation(
                            pt[:, :, c0:],
                            psS[:, :, c0:],
                            mybir.ActivationFunctionType.Exp,
                            scale=float(1.0 / np.sqrt(DH)),
                        )
                        if j >= 0:
                            # causal triangle mask on the diagonal 128-col
                            # window, as a DVE multiply (keeps gpsimd empty)
                            nc.vector.tensor_mul(
                                pt[:, :, c0:c0 + P],
                                pt[:, :, c0:c0 + P],
                                mask3[:],
                            )
                        pts.append((pt, c0))
                        if kv >= LAG:
                            pv_step(kv - LAG, stop=False)
                        if kv == 1 and pending:
                            pending[0]()
                        if kv == 3 and pending:
                            pending[1]()
                            pending = []
                        # front fills: one per cycle until exhausted (used
                        # for V-proj chains this block's own late PV needs)
                        if nfront < len(fill_front):
                            fill_front[nfront]()
                            nfront += 1
                        # weave fill chains at evenly spaced cycles
                        cyc += 1
                        want = cyc * len(fill) // ncycles
                        while nfill < want:
                            fill[nfill]()
                            nfill += 1
                    # per-pair PV tail, then ONE vector copy evacuates the
                    # unnormalized accumulator (both heads) to SBUF so the
                    # psum slot frees for the next pair
                    for kv in range(max(nkv - LAG, 0), nkv):
                        pv_step(kv, stop=(kv == nkv - 1))
                    last_pair = final and hp == HPC // 2 - 1
                    if not last_pair:
                        ou = spool.tile(
                            [DH + 1, 2, SB], f32, name="ou", tag="ou", bufs=2
                        )
                        nc.vector.tensor_copy(out=ou[:], in_=po[:])
                    else:
                        ou = po   # normalize straight from psum at the tail

                    # batched normalization for both heads of the pair, as
                    # two deferred stages (denominator staged through
                    # partition 0 - custom DVE ops break on nonzero base
                    # partitions on HW; recip/broadcast per head slot so the
                    # second broadcast overlaps the first mul)
                    def norm(ou, mo_, on_scalar):
                        dd = spool.tile(
                            [1, 2, SB], f32, name="dd", tag="dd", bufs=2
                        )
                        dinv = spool.tile(
                            [1, 2, SB], f32, name="dinv", tag="di", bufs=2
                        )
                        rb = spool.tile(
                            [DH, 2, SB], f32, name="rb", tag="rb", bufs=2
                        )

                        def stage1():
                            if on_scalar:
                                nc.scalar.copy(dd[:], ou[DH:DH + 1, :, :])
                            else:
                                nc.vector.tensor_copy(
                                    out=dd[:], in_=ou[DH:DH + 1, :, :]
                                )
                            for slot in (0, 1):
                                nc.vector.reciprocal_approx_fast(
                                    out=dinv[:, slot, :], in_=dd[:, slot, :]
                                )
                                nc.gpsimd.partition_broadcast(
                                    rb[:, slot, :], dinv[:, slot, :]
                                )

                        def stage2():
                            for slot in (0, 1):
                                nc.vector.tensor_mul(
                                    ot_t[slot * DH:(slot + 1) * DH, mo_, qsl],
                                    ou[0:DH, slot, :],
                                    rb[:, slot, :],
                                )
                        return [stage1, stage2]

                    pending = norm(ou, mo, on_scalar=last_pair)
                    if last_pair:
                        # flush leftover fills first (PE work), then run the
                        # chain now; dummy matmuls (emitted by the caller)
                        # keep the PE clock gate warm across it
                        while nfront < len(fill_front):
                            fill_front[nfront]()
                            nfront += 1
                        while nfill < len(fill):
                            fill[nfill]()
                            nfill += 1
                        pending[0]()
                        pending[1]()
                        pending = []
                while nfront < len(fill_front):
                    fill_front[nfront]()
                    nfront += 1
                while nfill < len(fill):
                    fill[nfill]()
                    nfill += 1
                return pending

            # fill balancing: attn(1)/attn(2) are PE-bound under their
            # projection fills while attn(3) (the longest exp stream) has
            # PE slack, so every deferrable chain - all three earlier
            # output projections and block 3's V projection - weaves into
            # attn(3). vv(3) is front-loaded (one chain per cycle) because
            # attn(3)'s own PV consumes v_t[12..15] from cycle 12 on.
            qk0, vv0 = proj_chains(0)
            for c in qk0:
                c()
            for c in vv0[:4]:
                c()
            qk1, vv1 = proj_chains(1)
            qk2, vv2 = proj_chains(2)
            qk3, vv3 = proj_chains(3)
            fills = [
                vv0[4:] + qk1 + vv1,                  # attn(0)
                qk2,                                  # attn(1)
                qk3,                                  # attn(2)
                ph3_chains(0, alternate=False)
                + ph3_chains(1, alternate=False)
                + ph3_chains(2, alternate=False),     # attn(3)
            ]
            fronts = [(), (), vv2, vv3]
            carry = []
            for qb in range(NSB):
                carry = attn(qb, fills[qb], carry, fill_front=fronts[qb])
            # final norm chain runs on DVE/gpsimd; dummy matmuls keep the
            # PE HAM clock gate warm across it so the tail output
            # projection runs at full clock
            for i in range(NDUM):
                wdum = pspool.tile([P, SB], f32, name="wdum", tag="s", bufs=2)
                nc.tensor.matmul(
                    wdum[:], warm_w[:, 0:P], warm_w[:], start=True, stop=True
                )
            for c in ph3_chains(NSB - 1, alternate=True, use_o_tag=True):
                c()

    nc.compile()
    return nc


def _get_nc(s=S):
    if s not in _nc_cache:
        _nc_cache[s] = _build(s)
    return _nc_cache[s]


def make_in_maps(in_features, Wq, Wk, Wv, Wo):
    """Shard full inputs into 8 per-core input dicts (bf16 operands).

    All operands are pre-tiled into [partition=128, ...] layouts matching
    the kernel's SBUF tiles, so every device DMA is contiguous per
    partition:
      xt  [128, NSB, KC, 512]: xt[p, sb, k, j]  = X[b].T[128k+p, 512sb+j]
      wqt [128, KC, 256]:      wqt[p, k, f]     = Wq[rows].T[128k+p, f]
      wot [128, MC, 1024]:     wot[p, o, d]     = Wo[:, cols].T[128o+p, d]
    """
    import ml_dtypes
    bf = ml_dtypes.bfloat16
    x = np.asarray(in_features, dtype=np.float32)
    wq = np.asarray(Wq, dtype=np.float32)
    wk = np.asarray(Wk, dtype=np.float32)
    wv = np.asarray(Wv, dtype=np.float32)
    wo = np.asarray(Wo, dtype=np.float32)

    KC = D // 128
    NSB = S // 512

    def ptile(a, inner):
        # [D_like, inner_total] -> [128, D_like//128, inner...] p-major
        o = a.shape[0] // 128
        return np.ascontiguousarray(
            a.reshape(o, 128, *inner).transpose(1, 0, *range(2, 2 + len(inner)))
        ).astype(bf)

    xts = []
    for b in range(B):
        a = x[b].T  # [D, S]
        # [128, NSB, KC, 512]
        a = a.reshape(KC, 128, NSB, 512).transpose(1, 2, 0, 3)
        xts.append(np.ascontiguousarray(a).astype(bf))
    in_maps = []
    for c in range(NCORES):
        b, g = divmod(c, GROUPS)
        rows = slice(g * F, (g + 1) * F)
        in_maps.append(
            {
                "xt": xts[b],
                "wqt": ptile(wq[rows, :].T, [F]),
                "wkt": ptile(wk[rows, :].T, [F]),
                "wvt": ptile(wv[rows, :].T, [F]),
                "wot": ptile(wo[:, rows].T, [D]),
            }
        )
    return in_maps


def combine_outputs(results):
    """Sum the 4 partial Y per batch element back into [B, S, D]."""
    out = np.zeros((B, S, D), dtype=np.float32)
    for c in range(NCORES):
        b = c // GROUPS
        out[b] += np.asarray(results[c]["y"]).astype(np.float32)
    return out


def kernel(in_features, Wq, Wk, Wv, Wo):
    from concourse import bass_utils

    nc = _get_nc()
    in_maps = make_in_maps(in_features, Wq, Wk, Wv, Wo)
    res = bass_utils.run_bass_kernel_spmd(nc, in_maps, core_ids=list(range(NCORES)))
    return combine_outputs(res.results)


# revision 27
# speedup vs baseline: 1.0608x; 1.0608x over previous
"""Causal multi-head self-attention on 8 Trainium2 NeuronCores.

Sharding: core c = (b, g) with b = c // 4 (batch), g = c % 4 (head group).
Each core computes 4 of the 16 heads for one batch element:
  Q/K/V projections for feature rows 256g:256g+256 (Megatron column split),
  causal attention for those heads, and a partial output projection
  against Wo[:, 256g:256g+256] (row split). Host sums the 4 partials per batch.

All operands are pre-transposed on the host so the kernel never transposes:
  xt  = X[b].T          [D, S]   (d on partitions -> matmul contraction dim)
  wqt = Wq[rows].T      [D, 256]
  wkt = Wk[rows].T      [D, 256]
  wvt = Wv[rows].T      [D, 256]
  wot = Wo[:, cols].T   [256, D]

Attention is computed with scores transposed (S^T = K Q^T, kv on partitions)
so the PV matmul needs no transpose, and a ones-row appended to V yields the
softmax denominator inside the same accumulation.

v4 structure (changes from v3):
  - causal diagonal trimming: for the diagonal kv chunk at block-relative
    position j, the scores matmuls, exp, and PV matmuls only cover
    q >= 128j (the fully-masked prefix is never computed or read).
  - the triangle mask is applied as a DVE multiply against a constant
    [128,2,128] 0/1 tile instead of a gpsimd affine_select: gpsimd now only
    runs the per-pair reciprocal broadcast, so its strict FIFO can no longer
    convoy the PE behind norm chains (v3 lost ~12us + two HAM re-throttles
    to this).
  - softmax normalization is batched per head-pair: PV accumulates both
    heads into one [65,2,512] psum (2 banks), a single evac / denominator
    copy / reciprocal / partition_broadcast covers both heads, then two
    muls write ot_t. Denominator staged through partition 0 (custom DVE
    ops break on nonzero base partitions on HW).
  - psum budget: scores 2x2 banks double-buffered, po 2 banks single, proj
    [128,512] double-buffered (projection/output chains no longer stall on
    a single bank between chain and evac).
  - dummy warm matmuls cover the final norm chain so the tail output
    projection does not run at the HAM-throttled 1.2 GHz clock.
  - partial outputs are stored and DMA'd as fp16 (host sums in fp32),
    halving output traffic and the post-compute DMA tail.
"""

import sys

sys.path.insert(0, "/opt/trn_rl_repo")

import numpy as np

B = 2
S = 2048
D = 1024
H = 16
DH = 64

NCORES = 8
GROUPS = 4            # head groups (cores per batch element)
HPC = H // GROUPS     # heads per core = 4
F = HPC * DH          # feature slice per core = 256

_nc_cache = {}


def _build(s=S):
    import concourse.bass as bass  # noqa: F401
    import concourse.mybir as mybir
    import concourse.tile as tile
    from concourse import bacc

    f32 = mybir.dt.float32
    f16 = mybir.dt.float16
    bf16 = mybir.dt.bfloat16
    dmm = bf16  # matmul operand dtype

    P = 128
    SB = 512               # q-block / free-dim block
    NSB = s // SB          # q blocks
    KC = D // P            # 8 contraction chunks over D
    KH = KC // 2           # half-chain k chunks
    MC = F // P            # 2 feature chunks per core
    NSC = s // P           # s chunks of 128
    ND = D // SB           # 2 output column blocks
    NWARM = 20             # PE warmup matmuls during initial DMA wait
    NDUM = 18              # PE keep-warm matmuls over the final norm chain
    LAG = 2                # kv chunks the PV matmul trails the exp by

    nc = bacc.Bacc("TRN2", debug=False, num_devices=NCORES)
    # all inputs are pre-tiled on the host into [partition, ...] layouts so
    # every DMA reads contiguous 2-8KB per partition (the natural
    # rearrange patterns gave 0.5-1KB scattered segments and only
    # ~120-200 GB/s, starving the projection phase)
    xt = nc.dram_tensor("xt", [P, NSB, KC, SB], dmm, kind="ExternalInput").ap()
    wqt = nc.dram_tensor("wqt", [P, KC, F], dmm, kind="ExternalInput").ap()
    wkt = nc.dram_tensor("wkt", [P, KC, F], dmm, kind="ExternalInput").ap()
    wvt = nc.dram_tensor("wvt", [P, KC, F], dmm, kind="ExternalInput").ap()
    wot = nc.dram_tensor("wot", [P, MC, D], dmm, kind="ExternalInput").ap()
    y = nc.dram_tensor("y", [s, D], f16, kind="ExternalOutput").ap()

    with tile.TileContext(nc) as tc:
        with (
            tc.tile_pool(name="w", bufs=1) as wpool,
            tc.tile_pool(name="const", bufs=1) as cpool,
            tc.tile_pool(name="xt", bufs=2) as xpool,
            tc.tile_pool(name="qkv", bufs=1) as qkvpool,
            tc.tile_pool(name="pt", bufs=4) as ptpool,
            tc.tile_pool(name="small", bufs=4) as spool,
            tc.tile_pool(name="yst", bufs=3) as ypool,
            tc.tile_pool(name="ps", bufs=1, space="PSUM") as pspool,
        ):
            wq_s = wpool.tile([P, KC, F], dmm, name="wq_s")
            wk_s = wpool.tile([P, KC, F], dmm, name="wk_s")
            wv_s = wpool.tile([P, KC, F], dmm, name="wv_s")
            wo_s = wpool.tile([P, MC, D], dmm, name="wo_s")
            wqt_r = wqt
            wkt_r = wkt
            # all input DMAs posted upfront, in first-needed order, so a
            # late-posted transfer can never stall the in-order PE queue
            xt_tiles = [
                xpool.tile([P, KC, SB], dmm, name="xt_tile", bufs=4)
                for _ in range(NSB)
            ]
            # warm_w memset first on gpsimd so PE warmup starts the moment
            # the prologue ends (nothing queued ahead of it)
            warm_w = cpool.tile([P, SB], dmm, name="warm_w")
            nc.gpsimd.memset(warm_w[:], 0.0)

            # input DMAs on the sync queue in first-needed order; the two
            # late xt tiles ride the scalar queue so they never delay wo
            nc.sync.dma_start(wq_s[:, 0:KH, :], wqt_r[:, 0:KH, :])
            nc.sync.dma_start(wk_s[:, 0:KH, :], wkt_r[:, 0:KH, :])
            nc.sync.dma_start(xt_tiles[0][:, 0:KH, :], xt[:, 0, 0:KH, :])
            nc.sync.dma_start(wq_s[:, KH:KC, :], wqt_r[:, KH:KC, :])
            nc.sync.dma_start(wk_s[:, KH:KC, :], wkt_r[:, KH:KC, :])
            nc.sync.dma_start(xt_tiles[0][:, KH:KC, :], xt[:, 0, KH:KC, :])
            nc.sync.dma_start(wv_s[:], wvt[:])
            nc.sync.dma_start(xt_tiles[1][:], xt[:, 1, :, :])
            # wo is first needed by ph3(0), woven into attn(1) - well after
            # xt1 (which gates proj(1) right behind proj(0))
            nc.sync.dma_start(wo_s[:], wot[:])
            # xt2/xt3 stay on the same queue AFTER everything critical:
            # queues share HBM bandwidth, so a parallel queue would steal
            # bandwidth from the startup-critical transfers above
            for sb in range(2, NSB):
                nc.sync.dma_start(
                    xt_tiles[sb][:], xt[:, sb, :, :]
                )

            # --- PE warmup: dummy matmuls on memset data so the HAM clock
            # gate reaches 8/8 before the first real (DMA-gated) matmul ---
            for i in range(NWARM):
                wps = pspool.tile([P, SB], f32, name="wps", tag="proj", bufs=2)
                nc.tensor.matmul(
                    wps[:], warm_w[:, 0:P], warm_w[:], start=True, stop=True
                )

            # --- constant triangle mask: mask3[r, slot, c] = 1 iff c >= r.
            # Applied to the diagonal 128-col window of pt by a DVE multiply
            # (stale columns left of the window are never read). ---
            mask3 = cpool.tile([P, 2, P], dmm, name="mask3")
            nc.gpsimd.memset(mask3[:], 1.0)
            nc.gpsimd.affine_select(
                out=mask3[:],
                in_=mask3[:],
                compare_op=mybir.AluOpType.is_ge,
                fill=0.0,
                base=0,
                pattern=[[0, 2], [1, P]],
                channel_multiplier=-1,
            )

            # --- persistent activations ---
            qt_t = qkvpool.tile([P, MC, s], dmm, name="qt_t")   # Q^T
            kt_t = qkvpool.tile([P, MC, s], dmm, name="kt_t")   # K^T
            v_t = qkvpool.tile([P, NSC, HPC, DH + 1], dmm, name="v_t")  # V | 1
            ot_t = qkvpool.tile([P, MC, s], dmm, name="ot_t")   # attn out ^T
            ones_sb = cpool.tile([P, NSC * HPC], f32, name="ones_sb")
            nc.gpsimd.memset(ones_sb[:], 1.0)
            nc.vector.tensor_copy(
                out=v_t[:, :, :, DH:DH + 1],
                in_=ones_sb.rearrange("p (a b) -> p a b", b=HPC)[:, :, :, None],
            )

            copy_alt = [0]

            def evac(dst, src, alternate):
                """psum -> sbuf evacuation, optionally alternating engines."""
                if alternate:
                    copy_alt[0] ^= 1
                    if copy_alt[0]:
                        nc.scalar.copy(dst, src)
                        return
                nc.vector.tensor_copy(out=dst, in_=src)

            def proj_chains(sb, alternate=False):
                """Projection block sb as a list of half-chain closures."""
                xt_tile = xt_tiles[sb]

                def qk_half(w_s, dst, m, kh, box):
                    def emit():
                        if kh == 0:
                            box["pp"] = pspool.tile(
                                [P, SB], f32, name="pp", tag="proj", bufs=2
                            )
                        pp = box["pp"]
                        for k in range(kh * KH, (kh + 1) * KH):
                            nc.tensor.matmul(
                                pp[:],
                                (w_s[:, k, m * P:(m + 1) * P]),
                                (xt_tile[:, k, :]),
                                start=(k == 0),
                                stop=(k == KC - 1),
                            )
                        if kh == 1:
                            evac(
                                dst[:, m, sb * SB:(sb + 1) * SB], pp[:],
                                alternate,
                            )
                    return emit

                def v_half(sc, kh, box):
                    def emit():
                        if kh == 0:
                            box["pv"] = pspool.tile(
                                [P, SB], f32, name="pv", tag="proj", bufs=2
                            )
                        pv = box["pv"]
                        for k in range(kh * KH, (kh + 1) * KH):
                            nc.tensor.matmul(
                                pv[:, :F],
                                (xt_tile[:, k, sc * P:(sc + 1) * P]),
                                (wv_s[:, k, :]),
                                start=(k == 0),
                                stop=(k == KC - 1),
                            )
                        if kh == 1:
                            nc.vector.tensor_copy(
                                out=v_t[:, sb * 4 + sc, :, 0:DH],
                                in_=pv[:, :F].rearrange(
                                    "p (h d) -> p h d", d=DH
                                ),
                            )
                    return emit

                qk = []
                for m in range(MC):
                    for w_s, dst in ((wq_s, qt_t), (wk_s, kt_t)):
                        box = {}
                        qk.append(qk_half(w_s, dst, m, 0, box))
                        qk.append(qk_half(w_s, dst, m, 1, box))
                vv = []
                for sc in range(SB // P):
                    box = {}
                    vv.append(v_half(sc, 0, box))
                    vv.append(v_half(sc, 1, box))
                return qk, vv

            def ph3_chains(qb, alternate, use_o_tag=False):
                """Output projection for q-block qb as chain closures. Both
                column blocks of a q-chunk evacuate into one [128, 1024] ys
                tile and ship as a single DMA (the ~600ns/instruction DMA
                issue rate on the sync queue otherwise throttles the tail).
                In the final block py rotates across the freed proj+s psum
                tags (4 banks) so the chains never wait on an evac."""
                def chain(sc, nb, box):
                    tag, bufs = ("s", 2) if use_o_tag and (sc * ND + nb) % 2 \
                        else ("proj", 2)

                    def emit():
                        py = pspool.tile(
                            [P, SB], f32, name="py", tag=tag, bufs=bufs
                        )
                        for o in range(MC):
                            nc.tensor.matmul(
                                py[:],
                                (ot_t[:, o, sc * P:(sc + 1) * P]),
                                (wo_s[:, o, nb * SB:(nb + 1) * SB]),
                                start=(o == 0),
                                stop=(o == MC - 1),
                            )
                        if nb == 0:
                            box["ys"] = ypool.tile(
                                [P, ND, SB], f16, name="ys", bufs=4
                            )
                        ys = box["ys"]
                        evac(ys[:, nb, :], py[:], alternate)
                        if nb == ND - 1:
                            nc.sync.dma_start(
                                y[sc * P:(sc + 1) * P, :],
                                ys[:],
                            )
                    return emit

                out = []
                for sc in range(4 * qb, 4 * qb + 4):
                    box = {}
                    for nb in range(ND):
                        out.append(chain(sc, nb, box))
                return out

            def attn(qb, fill, pre_norms, fill_front=()):
                """Attention for q-block qb; fill chains are woven into the
                exp-paced kv loop to keep the in-order PE queue busy.
                pre_norms are the deferred normalize stages of the previous
                head-pair/q-block: stage 1 (denominator copy + reciprocal +
                broadcast) fires at kv==1, stage 2 (the muls, which wait on
                the gpsimd broadcast) at kv==3 - so this block's mask
                multiplies are never queued in the DVE FIFO behind a mul
                that is itself blocked on gpsimd."""
                nkv = 4 * (qb + 1)
                final = qb == NSB - 1
                qsl = slice(qb * SB, (qb + 1) * SB)
                ncycles = (HPC // 2) * nkv
                cyc = 0
                nfill = 0
                nfront = 0
                pending = list(pre_norms)
                for hp in range(HPC // 2):
                    mo = hp
                    # both heads of the pair accumulate into one psum tile
                    # (2 banks); a single evac/recip/broadcast then serves
                    # both heads' normalization
                    po = pspool.tile(
                        [DH + 1, 2, SB], f32, name="po", tag="o", bufs=1
                    )
                    pts = []   # (pt tile, c0) per kv chunk
                    c0s = []

                    def pv_step(kv, stop):
                        pt, c0 = pts[kv]
                        for slot in (0, 1):
                            h = 2 * hp + slot
                            nc.tensor.matmul(
                                po[:, slot, c0:],
                                (v_t[:, kv, h, :]),
                                (pt[:, slot, c0:]),
                                start=(kv == 0),
                                stop=stop,
                            )

                    for kv in range(nkv):
                        j = kv - 4 * qb
                        c0 = P * j if j > 0 else 0
                        qs0 = qb * SB + c0
                        psS = pspool.tile(
                            [P, 2, SB], f32, name="psS", tag="s", bufs=2
                        )
                        # the two heads of the pair sit on different PE row
                        # groups (rows 0-63 / 64-127) -> concurrent matmuls
                        nc.tensor.matmul(
                            psS[:, 0, c0:],
                            (kt_t[0:DH, mo, kv * P:(kv + 1) * P]),
                            (qt_t[0:DH, mo, qs0:(qb + 1) * SB]),
                            start=True,
                            stop=True,
                        )
                        nc.tensor.matmul(
                            psS[:, 1, c0:],
                            (kt_t[DH:P, mo, kv * P:(kv + 1) * P]),
                            (qt_t[DH:P, mo, qs0:(qb + 1) * SB]),
                            start=True,
                            stop=True,
                        )
                        pt = ptpool.tile([P, 2, SB], dmm, name="pt", bufs=4)
                        nc.scalar.activation(
                            pt[:, :, c0:],
                            psS[:, :, c0:],
                            mybir.ActivationFunctionType.Exp,
                            scale=float(1.0 / np.sqrt(DH)),
                        )
                        if j >= 0:
                            # causal triangle mask on the diagonal 128-col
                            # window, as a DVE multiply (keeps gpsimd empty)
                            nc.vector.tensor_mul(
                                pt[:, :, c0:c0 + P],
                                pt[:, :, c0:c0 + P],
                                mask3[:],
                            )
                        pts.append((pt, c0))
                        if kv >= LAG:
                            pv_step(kv - LAG, stop=False)
                        if kv == 1 and pending:
                            pending[0]()
                        if kv == 3 and pending:
                            pending[1]()
                            pending = []
                        # front fills: one per cycle until exhausted (used
                        # for V-proj chains this block's own late PV needs)
                        if nfront < len(fill_front):
                            fill_front[nfront]()
                            nfront += 1
                        # weave fill chains at evenly spaced cycles
                        cyc += 1
                        want = cyc * len(fill) // ncycles
                        while nfill < want:
                            fill[nfill]()
                            nfill += 1
                    # per-pair PV tail, then ONE vector copy evacuates the
                    # unnormalized accumulator (both heads) to SBUF so the
                    # psum slot frees for the next pair
                    for kv in range(max(nkv - LAG, 0), nkv):
                        pv_step(kv, stop=(kv == nkv - 1))
                    last_pair = final and hp == HPC // 2 - 1
                    if not last_pair:
                        ou = spool.tile(
                            [DH + 1, 2, SB], f32, name="ou", tag="ou", bufs=2
                        )
                        nc.vector.tensor_copy(out=ou[:], in_=po[:])
                    else:
                        ou = po   # normalize straight from psum at the tail

                    # batched normalization for both heads of the pair, as
                    # two deferred stages (denominator staged through
                    # partition 0 - custom DVE ops break on nonzero base
                    # partitions on HW; recip/broadcast per head slot so the
                    # second broadcast overlaps the first mul)
                    def norm(ou, mo_, on_scalar):
                        dd = spool.tile(
                            [1, 2, SB], f32, name="dd", tag="dd", bufs=2
                        )
                        dinv = spool.tile(
                            [1, 2, SB], f32, name="dinv", tag="di", bufs=2
                        )
                        rb = spool.tile(
                            [DH, 2, SB], f32, name="rb", tag="rb", bufs=2
                        )

                        def stage1():
                            if on_scalar:
                                nc.scalar.copy(dd[:], ou[DH:DH + 1, :, :])
                            else:
                                nc.vector.tensor_copy(
                                    out=dd[:], in_=ou[DH:DH + 1, :, :]
                                )
                            for slot in (0, 1):
                                nc.vector.reciprocal_approx_fast(
                                    out=dinv[:, slot, :], in_=dd[:, slot, :]
                                )
                                nc.gpsimd.partition_broadcast(
                                    rb[:, slot, :], dinv[:, slot, :]
                                )

                        def stage2():
                            for slot in (0, 1):
                                nc.vector.tensor_mul(
                                    ot_t[slot * DH:(slot + 1) * DH, mo_, qsl],
                                    ou[0:DH, slot, :],
                                    rb[:, slot, :],
                                )
                        return [stage1, stage2]

                    pending = norm(ou, mo, on_scalar=last_pair)
                    if last_pair:
                        # flush leftover fills first (PE work), then run the
                        # chain now; dummy matmuls (emitted by the caller)
                        # keep the PE clock gate warm across it
                        while nfront < len(fill_front):
                            fill_front[nfront]()
                            nfront += 1
                        while nfill < len(fill):
                            fill[nfill]()
                            nfill += 1
                        pending[0]()
                        pending[1]()
                        pending = []
                while nfront < len(fill_front):
                    fill_front[nfront]()
                    nfront += 1
                while nfill < len(fill):
                    fill[nfill]()
                    nfill += 1
                return pending

            # fill balancing: attn(1)/attn(2) are PE-bound under their
            # projection fills while attn(3) (the longest exp stream) has
            # PE slack, so every deferrable chain - all three earlier
            # output projections and block 3's V projection - weaves into
            # attn(3). vv(3) is front-loaded (one chain per cycle) because
            # attn(3)'s own PV consumes v_t[12..15] from cycle 12 on.
            qk0, vv0 = proj_chains(0)
            for c in qk0:
                c()
            for c in vv0[:4]:
                c()
            qk1, vv1 = proj_chains(1)
            qk2, vv2 = proj_chains(2)
            qk3, vv3 = proj_chains(3)
            fills = [
                vv0[4:] + qk1 + vv1,                  # attn(0)
                qk2,                                  # attn(1)
                qk3 + vv3,                            # attn(2)
                ph3_chains(0, alternate=False)
                + ph3_chains(1, alternate=False)
                + ph3_chains(2, alternate=False),     # attn(3)
            ]
            fronts = [(), (), vv2, ()]
            carry = []
            for qb in range(NSB):
                carry = attn(qb, fills[qb], carry, fill_front=fronts[qb])
            # final norm chain runs on DVE/gpsimd; dummy matmuls keep the
            # PE HAM clock gate warm across it so the tail output
            # projection runs at full clock
            for i in range(NDUM):
                wdum = pspool.tile([P, SB], f32, name="wdum", tag="s", bufs=2)
                nc.tensor.matmul(
                    wdum[:], warm_w[:, 0:P], warm_w[:], start=True, stop=True
                )
            for c in ph3_chains(NSB - 1, alternate=True, use_o_tag=True):
                c()

    nc.compile()
    return nc


def _get_nc(s=S):
    if s not in _nc_cache:
        _nc_cache[s] = _build(s)
    return _nc_cache[s]


def make_in_maps(in_features, Wq, Wk, Wv, Wo):
    """Shard full inputs into 8 per-core input dicts (bf16 operands).

    All operands are pre-tiled into [partition=128, ...] layouts matching
    the kernel's SBUF tiles, so every device DMA is contiguous per
    partition:
      xt  [128, NSB, KC, 512]: xt[p, sb, k, j]  = X[b].T[128k+p, 512sb+j]
      wqt [128, KC, 256]:      wqt[p, k, f]     = Wq[rows].T[128k+p, f]
      wot [128, MC, 1024]:     wot[p, o, d]     = Wo[:, cols].T[128o+p, d]
    """
    import ml_dtypes
    bf = ml_dtypes.bfloat16
    x = np.asarray(in_features, dtype=np.float32)
    wq = np.asarray(Wq, dtype=np.float32)
    wk = np.asarray(Wk, dtype=np.float32)
    wv = np.asarray(Wv, dtype=np.float32)
    wo = np.asarray(Wo, dtype=np.float32)

    KC = D // 128
    NSB = S // 512

    def ptile(a, inner):
        # [D_like, inner_total] -> [128, D_like//128, inner...] p-major
        o = a.shape[0] // 128
        return np.ascontiguousarray(
            a.reshape(o, 128, *inner).transpose(1, 0, *range(2, 2 + len(inner)))
        ).astype(bf)

    xts = []
    for b in range(B):
        a = x[b].T  # [D, S]
        # [128, NSB, KC, 512]
        a = a.reshape(KC, 128, NSB, 512).transpose(1, 2, 0, 3)
        xts.append(np.ascontiguousarray(a).astype(bf))
    in_maps = []
    for c in range(NCORES):
        b, g = divmod(c, GROUPS)
        rows = slice(g * F, (g + 1) * F)
        in_maps.append(
            {
                "xt": xts[b],
                "wqt": ptile(wq[rows, :].T, [F]),
                "wkt": ptile(wk[rows, :].T, [F]),
                "wvt": ptile(wv[rows, :].T, [F]),
                "wot": ptile(wo[:, rows].T, [D]),
            }
        )
    return in_maps


def combine_outputs(results):
    """Sum the 4 partial Y per batch element back into [B, S, D]."""
    out = np.zeros((B, S, D), dtype=np.float32)
    for c in range(NCORES):
        b = c // GROUPS
        out[b] += np.asarray(results[c]["y"]).astype(np.float32)
    return out


def kernel(in_features, Wq, Wk, Wv, Wo):
    from concourse import bass_utils

    nc = _get_nc()
    in_maps = make_in_maps(in_features, Wq, Wk, Wv, Wo)
    res = bass_utils.run_bass_kernel_spmd(nc, in_maps, core_ids=list(range(NCORES)))
    return combine_outputs(res.results)


# revision 30
# speedup vs baseline: 1.0630x; 1.0021x over previous
"""Causal multi-head self-attention on 8 Trainium2 NeuronCores.

Sharding: core c = (b, g) with b = c // 4 (batch), g = c % 4 (head group).
Each core computes 4 of the 16 heads for one batch element:
  Q/K/V projections for feature rows 256g:256g+256 (Megatron column split),
  causal attention for those heads, and a partial output projection
  against Wo[:, 256g:256g+256] (row split). Host sums the 4 partials per batch.

All operands are pre-transposed on the host so the kernel never transposes:
  xt  = X[b].T          [D, S]   (d on partitions -> matmul contraction dim)
  wqt = Wq[rows].T      [D, 256]
  wkt = Wk[rows].T      [D, 256]
  wvt = Wv[rows].T      [D, 256]
  wot = Wo[:, cols].T   [256, D]

Attention is computed with scores transposed (S^T = K Q^T, kv on partitions)
so the PV matmul needs no transpose, and a ones-row appended to V yields the
softmax denominator inside the same accumulation.

v4 structure (changes from v3):
  - causal diagonal trimming: for the diagonal kv chunk at block-relative
    position j, the scores matmuls, exp, and PV matmuls only cover
    q >= 128j (the fully-masked prefix is never computed or read).
  - the triangle mask is applied as a DVE multiply against a constant
    [128,2,128] 0/1 tile instead of a gpsimd affine_select: gpsimd now only
    runs the per-pair reciprocal broadcast, so its strict FIFO can no longer
    convoy the PE behind norm chains (v3 lost ~12us + two HAM re-throttles
    to this).
  - softmax normalization is batched per head-pair: PV accumulates both
    heads into one [65,2,512] psum (2 banks), a single evac / denominator
    copy / reciprocal / partition_broadcast covers both heads, then two
    muls write ot_t. Denominator staged through partition 0 (custom DVE
    ops break on nonzero base partitions on HW).
  - psum budget: scores 2x2 banks double-buffered, po 2 banks single, proj
    [128,512] double-buffered (projection/output chains no longer stall on
    a single bank between chain and evac).
  - dummy warm matmuls cover the final norm chain so the tail output
    projection does not run at the HAM-throttled 1.2 GHz clock.
  - partial outputs are stored and DMA'd as fp16 (host sums in fp32),
    halving output traffic and the post-compute DMA tail.
"""

import sys

sys.path.insert(0, "/opt/trn_rl_repo")

import numpy as np

B = 2
S = 2048
D = 1024
H = 16
DH = 64

NCORES = 8
GROUPS = 4            # head groups (cores per batch element)
HPC = H // GROUPS     # heads per core = 4
F = HPC * DH          # feature slice per core = 256

_nc_cache = {}


def _build(s=S):
    import concourse.bass as bass  # noqa: F401
    import concourse.mybir as mybir
    import concourse.tile as tile
    from concourse import bacc

    f32 = mybir.dt.float32
    f16 = mybir.dt.float16
    bf16 = mybir.dt.bfloat16
    dmm = bf16  # matmul operand dtype

    P = 128
    SB = 512               # q-block / free-dim block
    NSB = s // SB          # q blocks
    KC = D // P            # 8 contraction chunks over D
    KH = KC // 2           # half-chain k chunks
    MC = F // P            # 2 feature chunks per core
    NSC = s // P           # s chunks of 128
    ND = D // SB           # 2 output column blocks
    NWARM = 20             # PE warmup matmuls during initial DMA wait
    NDUM = 18              # PE keep-warm matmuls over the final norm chain
    LAG = 2                # kv chunks the PV matmul trails the exp by

    nc = bacc.Bacc("TRN2", debug=False, num_devices=NCORES)
    # all inputs are pre-tiled on the host into [partition, ...] layouts so
    # every DMA reads contiguous 2-8KB per partition (the natural
    # rearrange patterns gave 0.5-1KB scattered segments and only
    # ~120-200 GB/s, starving the projection phase)
    xt = nc.dram_tensor("xt", [P, NSB, KC, SB], dmm, kind="ExternalInput").ap()
    wqt = nc.dram_tensor("wqt", [P, KC, F], dmm, kind="ExternalInput").ap()
    wkt = nc.dram_tensor("wkt", [P, KC, F], dmm, kind="ExternalInput").ap()
    wvt = nc.dram_tensor("wvt", [P, KC, F], dmm, kind="ExternalInput").ap()
    wot = nc.dram_tensor("wot", [P, MC, D], dmm, kind="ExternalInput").ap()
    y = nc.dram_tensor("y", [s, D], f16, kind="ExternalOutput").ap()

    with tile.TileContext(nc) as tc:
        with (
            tc.tile_pool(name="w", bufs=1) as wpool,
            tc.tile_pool(name="const", bufs=1) as cpool,
            tc.tile_pool(name="xt", bufs=2) as xpool,
            tc.tile_pool(name="qkv", bufs=1) as qkvpool,
            tc.tile_pool(name="pt", bufs=4) as ptpool,
            tc.tile_pool(name="small", bufs=4) as spool,
            tc.tile_pool(name="yst", bufs=3) as ypool,
            tc.tile_pool(name="ps", bufs=1, space="PSUM") as pspool,
        ):
            wq_s = wpool.tile([P, KC, F], dmm, name="wq_s")
            wk_s = wpool.tile([P, KC, F], dmm, name="wk_s")
            wv_s = wpool.tile([P, KC, F], dmm, name="wv_s")
            wo_s = wpool.tile([P, MC, D], dmm, name="wo_s")
            wqt_r = wqt
            wkt_r = wkt
            # all input DMAs posted upfront, in first-needed order, so a
            # late-posted transfer can never stall the in-order PE queue
            xt_tiles = [
                xpool.tile([P, KC, SB], dmm, name="xt_tile", bufs=4)
                for _ in range(NSB)
            ]
            # warm_w memset first on gpsimd so PE warmup starts the moment
            # the prologue ends (nothing queued ahead of it)
            warm_w = cpool.tile([P, SB], dmm, name="warm_w")
            nc.gpsimd.memset(warm_w[:], 0.0)

            # input DMAs on the sync queue in first-needed order; the two
            # late xt tiles ride the scalar queue so they never delay wo
            nc.sync.dma_start(wq_s[:, 0:KH, :], wqt_r[:, 0:KH, :])
            nc.sync.dma_start(wk_s[:, 0:KH, :], wkt_r[:, 0:KH, :])
            nc.sync.dma_start(xt_tiles[0][:, 0:KH, :], xt[:, 0, 0:KH, :])
            nc.sync.dma_start(wq_s[:, KH:KC, :], wqt_r[:, KH:KC, :])
            nc.sync.dma_start(wk_s[:, KH:KC, :], wkt_r[:, KH:KC, :])
            nc.sync.dma_start(xt_tiles[0][:, KH:KC, :], xt[:, 0, KH:KC, :])
            nc.sync.dma_start(wv_s[:], wvt[:])
            nc.sync.dma_start(xt_tiles[1][:], xt[:, 1, :, :])
            # wo is first needed by ph3(0), woven into attn(1) - well after
            # xt1 (which gates proj(1) right behind proj(0))
            nc.sync.dma_start(wo_s[:], wot[:])
            # xt2/xt3 stay on the same queue AFTER everything critical:
            # queues share HBM bandwidth, so a parallel queue would steal
            # bandwidth from the startup-critical transfers above
            for sb in range(2, NSB):
                nc.sync.dma_start(
                    xt_tiles[sb][:], xt[:, sb, :, :]
                )

            # --- PE warmup: dummy matmuls on memset data so the HAM clock
            # gate reaches 8/8 before the first real (DMA-gated) matmul ---
            for i in range(NWARM):
                wps = pspool.tile([P, SB], f32, name="wps", tag="proj", bufs=2)
                nc.tensor.matmul(
                    wps[:], warm_w[:, 0:P], warm_w[:], start=True, stop=True
                )

            # --- constant triangle mask: mask3[r, slot, c] = 1 iff c >= r.
            # Applied to the diagonal 128-col window of pt by a DVE multiply
            # (stale columns left of the window are never read). ---
            mask3 = cpool.tile([P, 2, P], dmm, name="mask3")
            nc.gpsimd.memset(mask3[:], 1.0)
            nc.gpsimd.affine_select(
                out=mask3[:],
                in_=mask3[:],
                compare_op=mybir.AluOpType.is_ge,
                fill=0.0,
                base=0,
                pattern=[[0, 2], [1, P]],
                channel_multiplier=-1,
            )

            # --- persistent activations ---
            qt_t = qkvpool.tile([P, MC, s], dmm, name="qt_t")   # Q^T
            kt_t = qkvpool.tile([P, MC, s], dmm, name="kt_t")   # K^T
            v_t = qkvpool.tile([P, NSC, HPC, DH + 1], dmm, name="v_t")  # V | 1
            ot_t = qkvpool.tile([P, MC, s], dmm, name="ot_t")   # attn out ^T
            ones_sb = cpool.tile([P, NSC * HPC], f32, name="ones_sb")
            nc.gpsimd.memset(ones_sb[:], 1.0)
            nc.vector.tensor_copy(
                out=v_t[:, :, :, DH:DH + 1],
                in_=ones_sb.rearrange("p (a b) -> p a b", b=HPC)[:, :, :, None],
            )

            copy_alt = [0]

            def evac(dst, src, alternate):
                """psum -> sbuf evacuation, optionally alternating engines."""
                if alternate:
                    copy_alt[0] ^= 1
                    if copy_alt[0]:
                        nc.scalar.copy(dst, src)
                        return
                nc.vector.tensor_copy(out=dst, in_=src)

            def proj_chains(sb, alternate=False):
                """Projection block sb as a list of half-chain closures."""
                xt_tile = xt_tiles[sb]

                def qk_half(w_s, dst, m, kh, box, tag="proj"):
                    def emit():
                        if kh == 0:
                            box["pp"] = pspool.tile(
                                [P, SB], f32, name="pp", tag=tag, bufs=2
                            )
                        pp = box["pp"]
                        for k in range(kh * KH, (kh + 1) * KH):
                            nc.tensor.matmul(
                                pp[:],
                                (w_s[:, k, m * P:(m + 1) * P]),
                                (xt_tile[:, k, :]),
                                start=(k == 0),
                                stop=(k == KC - 1),
                            )
                        if kh == 1:
                            evac(
                                dst[:, m, sb * SB:(sb + 1) * SB], pp[:],
                                alternate,
                            )
                    return emit

                def v_half(sc, kh, box):
                    def emit():
                        if kh == 0:
                            box["pv"] = pspool.tile(
                                [P, SB], f32, name="pv", tag="proj", bufs=2
                            )
                        pv = box["pv"]
                        for k in range(kh * KH, (kh + 1) * KH):
                            nc.tensor.matmul(
                                pv[:, :F],
                                (xt_tile[:, k, sc * P:(sc + 1) * P]),
                                (wv_s[:, k, :]),
                                start=(k == 0),
                                stop=(k == KC - 1),
                            )
                        if kh == 1:
                            nc.vector.tensor_copy(
                                out=v_t[:, sb * 4 + sc, :, 0:DH],
                                in_=pv[:, :F].rearrange(
                                    "p (h d) -> p h d", d=DH
                                ),
                            )
                    return emit

                qk = []
                if sb == 0:
                    # startup block: all kh0 halves (xt0a-only work) first,
                    # the m=1 chains on the idle s tag, so four chains keep
                    # the PE busy while the xt0b transfer is in flight
                    boxes = {}
                    for m in range(MC):
                        for ti, (w_s, dst) in enumerate(
                            ((wq_s, qt_t), (wk_s, kt_t))
                        ):
                            boxes[m, ti] = {}
                            qk.append(qk_half(
                                w_s, dst, m, 0, boxes[m, ti],
                                tag="proj" if m == 0 else "s",
                            ))
                    for m in range(MC):
                        for ti, (w_s, dst) in enumerate(
                            ((wq_s, qt_t), (wk_s, kt_t))
                        ):
                            qk.append(qk_half(w_s, dst, m, 1, boxes[m, ti]))
                else:
                    for m in range(MC):
                        for w_s, dst in ((wq_s, qt_t), (wk_s, kt_t)):
                            box = {}
                            qk.append(qk_half(w_s, dst, m, 0, box))
                            qk.append(qk_half(w_s, dst, m, 1, box))
                vv = []
                for sc in range(SB // P):
                    box = {}
                    vv.append(v_half(sc, 0, box))
                    vv.append(v_half(sc, 1, box))
                return qk, vv

            def ph3_chains(qb, alternate, use_o_tag=False):
                """Output projection for q-block qb as chain closures. Both
                column blocks of a q-chunk evacuate into one [128, 1024] ys
                tile and ship as a single DMA (the ~600ns/instruction DMA
                issue rate on the sync queue otherwise throttles the tail).
                In the final block py rotates across the freed proj+s psum
                tags (4 banks) so the chains never wait on an evac."""
                def chain(sc, nb, box):
                    tag, bufs = ("s", 2) if use_o_tag and (sc * ND + nb) % 2 \
                        else ("proj", 2)

                    def emit():
                        py = pspool.tile(
                            [P, SB], f32, name="py", tag=tag, bufs=bufs
                        )
                        for o in range(MC):
                            nc.tensor.matmul(
                                py[:],
                                (ot_t[:, o, sc * P:(sc + 1) * P]),
                                (wo_s[:, o, nb * SB:(nb + 1) * SB]),
                                start=(o == 0),
                                stop=(o == MC - 1),
                            )
                        if nb == 0:
                            box["ys"] = ypool.tile(
                                [P, ND, SB], f16, name="ys", bufs=4
                            )
                        ys = box["ys"]
                        evac(ys[:, nb, :], py[:], alternate)
                        if nb == ND - 1:
                            nc.sync.dma_start(
                                y[sc * P:(sc + 1) * P, :],
                                ys[:],
                            )
                    return emit

                out = []
                for sc in range(4 * qb, 4 * qb + 4):
                    box = {}
                    for nb in range(ND):
                        out.append(chain(sc, nb, box))
                return out

            def attn(qb, fill, pre_norms, fill_front=()):
                """Attention for q-block qb; fill chains are woven into the
                exp-paced kv loop to keep the in-order PE queue busy.
                pre_norms are the deferred normalize stages of the previous
                head-pair/q-block: stage 1 (denominator copy + reciprocal +
                broadcast) fires at kv==1, stage 2 (the muls, which wait on
                the gpsimd broadcast) at kv==3 - so this block's mask
                multiplies are never queued in the DVE FIFO behind a mul
                that is itself blocked on gpsimd."""
                nkv = 4 * (qb + 1)
                final = qb == NSB - 1
                qsl = slice(qb * SB, (qb + 1) * SB)
                ncycles = (HPC // 2) * nkv
                cyc = 0
                nfill = 0
                nfront = 0
                pending = list(pre_norms)
                for hp in range(HPC // 2):
                    mo = hp
                    # both heads of the pair accumulate into one psum tile
                    # (2 banks); a single evac/recip/broadcast then serves
                    # both heads' normalization
                    po = pspool.tile(
                        [DH + 1, 2, SB], f32, name="po", tag="o", bufs=1
                    )
                    pts = []   # (pt tile, c0) per kv chunk
                    c0s = []

                    def pv_step(kv, stop):
                        pt, c0 = pts[kv]
                        for slot in (0, 1):
                            h = 2 * hp + slot
                            nc.tensor.matmul(
                                po[:, slot, c0:],
                                (v_t[:, kv, h, :]),
                                (pt[:, slot, c0:]),
                                start=(kv == 0),
                                stop=stop,
                            )

                    for kv in range(nkv):
                        j = kv - 4 * qb
                        c0 = P * j if j > 0 else 0
                        qs0 = qb * SB + c0
                        psS = pspool.tile(
                            [P, 2, SB], f32, name="psS", tag="s", bufs=2
                        )
                        # the two heads of the pair sit on different PE row
                        # groups (rows 0-63 / 64-127) -> concurrent matmuls
                        nc.tensor.matmul(
                            psS[:, 0, c0:],
                            (kt_t[0:DH, mo, kv * P:(kv + 1) * P]),
                            (qt_t[0:DH, mo, qs0:(qb + 1) * SB]),
                            start=True,
                            stop=True,
                        )
                        nc.tensor.matmul(
                            psS[:, 1, c0:],
                            (kt_t[DH:P, mo, kv * P:(kv + 1) * P]),
                            (qt_t[DH:P, mo, qs0:(qb + 1) * SB]),
                            start=True,
                            stop=True,
                        )
                        pt = ptpool.tile([P, 2, SB], dmm, name="pt", bufs=4)
                        nc.scalar.activation(
                            pt[:, :, c0:],
                            psS[:, :, c0:],
                            mybir.ActivationFunctionType.Exp,
                            scale=float(1.0 / np.sqrt(DH)),
                        )
                        if j >= 0:
                            # causal triangle mask on the diagonal 128-col
                            # window, as a DVE multiply (keeps gpsimd empty)
                            nc.vector.tensor_mul(
                                pt[:, :, c0:c0 + P],
                                pt[:, :, c0:c0 + P],
                                mask3[:],
                            )
                        pts.append((pt, c0))
                        if kv >= LAG:
                            pv_step(kv - LAG, stop=False)
                        if kv == 1 and pending:
                            pending[0]()
                        if kv == 3 and pending:
                            pending[1]()
                            pending = []
                        # front fills: one per cycle until exhausted (used
                        # for V-proj chains this block's own late PV needs)
                        if nfront < len(fill_front):
                            fill_front[nfront]()
                            nfront += 1
                        # weave fill chains at evenly spaced cycles
                        cyc += 1
                        want = cyc * len(fill) // ncycles
                        while nfill < want:
                            fill[nfill]()
                            nfill += 1
                    # per-pair PV tail, then ONE vector copy evacuates the
                    # unnormalized accumulator (both heads) to SBUF so the
                    # psum slot frees for the next pair
                    for kv in range(max(nkv - LAG, 0), nkv):
                        pv_step(kv, stop=(kv == nkv - 1))
                    last_pair = final and hp == HPC // 2 - 1
                    if not last_pair:
                        ou = spool.tile(
                            [DH + 1, 2, SB], f32, name="ou", tag="ou", bufs=2
                        )
                        nc.vector.tensor_copy(out=ou[:], in_=po[:])
                    else:
                        ou = po   # normalize straight from psum at the tail

                    # batched normalization for both heads of the pair, as
                    # two deferred stages (denominator staged through
                    # partition 0 - custom DVE ops break on nonzero base
                    # partitions on HW; recip/broadcast per head slot so the
                    # second broadcast overlaps the first mul)
                    def norm(ou, mo_, on_scalar):
                        dd = spool.tile(
                            [1, 2, SB], f32, name="dd", tag="dd", bufs=2
                        )
                        dinv = spool.tile(
                            [1, 2, SB], f32, name="dinv", tag="di", bufs=2
                        )
                        rb = spool.tile(
                            [DH, 2, SB], f32, name="rb", tag="rb", bufs=2
                        )

                        def stage1():
                            if on_scalar:
                                nc.scalar.copy(dd[:], ou[DH:DH + 1, :, :])
                            else:
                                nc.vector.tensor_copy(
                                    out=dd[:], in_=ou[DH:DH + 1, :, :]
                                )
                            for slot in (0, 1):
                                nc.vector.reciprocal_approx_fast(
                                    out=dinv[:, slot, :], in_=dd[:, slot, :]
                                )
                                nc.gpsimd.partition_broadcast(
                                    rb[:, slot, :], dinv[:, slot, :]
                                )

                        def stage2():
                            for slot in (0, 1):
                                nc.vector.tensor_mul(
                                    ot_t[slot * DH:(slot + 1) * DH, mo_, qsl],
                                    ou[0:DH, slot, :],
                                    rb[:, slot, :],
                                )
                        return [stage1, stage2]

                    pending = norm(ou, mo, on_scalar=last_pair)
                    if last_pair:
                        # flush leftover fills first (PE work), then run the
                        # chain now; dummy matmuls (emitted by the caller)
                        # keep the PE clock gate warm across it
                        while nfront < len(fill_front):
                            fill_front[nfront]()
                            nfront += 1
                        while nfill < len(fill):
                            fill[nfill]()
                            nfill += 1
                        pending[0]()
                        pending[1]()
                        pending = []
                while nfront < len(fill_front):
                    fill_front[nfront]()
                    nfront += 1
                while nfill < len(fill):
                    fill[nfill]()
                    nfill += 1
                return pending

            # fill balancing: attn(1)/attn(2) are PE-bound under their
            # projection fills while attn(3) (the longest exp stream) has
            # PE slack, so every deferrable chain - all three earlier
            # output projections and block 3's V projection - weaves into
            # attn(3). vv(3) is front-loaded (one chain per cycle) because
            # attn(3)'s own PV consumes v_t[12..15] from cycle 12 on.
            qk0, vv0 = proj_chains(0)
            for c in qk0:
                c()
            for c in vv0[:4]:
                c()
            qk1, vv1 = proj_chains(1)
            qk2, vv2 = proj_chains(2)
            qk3, vv3 = proj_chains(3)
            fills = [
                vv0[4:] + qk1 + vv1,                  # attn(0)
                qk2,                                  # attn(1)
                qk3,                                  # attn(2)
                vv3                                   # first: PV needs v_t
                + ph3_chains(0, alternate=False)
                + ph3_chains(1, alternate=False)
                + ph3_chains(2, alternate=False),     # attn(3)
            ]
            fronts = [(), (), vv2, ()]
            carry = []
            for qb in range(NSB):
                carry = attn(qb, fills[qb], carry, fill_front=fronts[qb])
            # final norm chain runs on DVE/gpsimd; dummy matmuls keep the
            # PE HAM clock gate warm across it so the tail output
            # projection runs at full clock
            for i in range(NDUM):
                wdum = pspool.tile([P, SB], f32, name="wdum", tag="s", bufs=2)
                nc.tensor.matmul(
                    wdum[:], warm_w[:, 0:P], warm_w[:], start=True, stop=True
                )
            for c in ph3_chains(NSB - 1, alternate=True, use_o_tag=True):
                c()

    nc.compile()
    return nc


def _get_nc(s=S):
    if s not in _nc_cache:
        _nc_cache[s] = _build(s)
    return _nc_cache[s]


def make_in_maps(in_features, Wq, Wk, Wv, Wo):
    """Shard full inputs into 8 per-core input dicts (bf16 operands).

    All operands are pre-tiled into [partition=128, ...] layouts matching
    the kernel's SBUF tiles, so every device DMA is contiguous per
    partition:
      xt  [128, NSB, KC, 512]: xt[p, sb, k, j]  = X[b].T[128k+p, 512sb+j]
      wqt [128, KC, 256]:      wqt[p, k, f]     = Wq[rows].T[128k+p, f]
      wot [128, MC, 1024]:     wot[p, o, d]     = Wo[:, cols].T[128o+p, d]
    """
    import ml_dtypes
    bf = ml_dtypes.bfloat16
    x = np.asarray(in_features, dtype=np.float32)
    wq = np.asarray(Wq, dtype=np.float32)
    wk = np.asarray(Wk, dtype=np.float32)
    wv = np.asarray(Wv, dtype=np.float32)
    wo = np.asarray(Wo, dtype=np.float32)

    KC = D // 128
    NSB = S // 512

    def ptile(a, inner):
        # [D_like, inner_total] -> [128, D_like//128, inner...] p-major
        o = a.shape[0] // 128
        return np.ascontiguousarray(
            a.reshape(o, 128, *inner).transpose(1, 0, *range(2, 2 + len(inner)))
        ).astype(bf)

    xts = []
    for b in range(B):
        a = x[b].T  # [D, S]
        # [128, NSB, KC, 512]
        a = a.reshape(KC, 128, NSB, 512).transpose(1, 2, 0, 3)
        xts.append(np.ascontiguousarray(a).astype(bf))
    in_maps = []
    for c in range(NCORES):
        b, g = divmod(c, GROUPS)
        rows = slice(g * F, (g + 1) * F)
        in_maps.append(
            {
                "xt": xts[b],
                "wqt": ptile(wq[rows, :].T, [F]),
                "wkt": ptile(wk[rows, :].T, [F]),
                "wvt": ptile(wv[rows, :].T, [F]),
                "wot": ptile(wo[:, rows].T, [D]),
            }
        )
    return in_maps


def combine_outputs(results):
    """Sum the 4 partial Y per batch element back into [B, S, D]."""
    out = np.zeros((B, S, D), dtype=np.float32)
    for c in range(NCORES):
        b = c // GROUPS
        out[b] += np.asarray(results[c]["y"]).astype(np.float32)
    return out


def kernel(in_features, Wq, Wk, Wv, Wo):
    from concourse import bass_utils

    nc = _get_nc()
    in_maps = make_in_maps(in_features, Wq, Wk, Wv, Wo)
    res = bass_utils.run_bass_kernel_spmd(nc, in_maps, core_ids=list(range(NCORES)))
    return combine_outputs(res.results)
